# revision 33
# baseline (speedup 1.0000x reference)
"""Trainium2 Bass kernel for nn_ConvTransduce1D (self-contained).

Computes, for x [16, 4096, 128] fp32, the CTC-style automaton forward scores
out [16, 4096, 52] of 52 tiny lexicon automata (26 single-token [c], 26
two-token [c, c+1], c = 1..26, blank = 0) over sliding windows of K=5 frames
(stride 1, pad 2).

Closed form (validated against the jax reference):
  For window w, padded frames give u_t = exp(x[w+t, c] - x[w+t, 0]) and
  v_t = exp(x[w+t, c+1] - x[w+t, 0]), t = 0..4; Sb = sum_t x[w+t, 0].
  e_t  = u_t * (1 + e_{t-1})        (runs of c ending at t;   e_0 = u_0)
  C_t  = C_{t-1} + e_t              (prefix sums;             C_0 = e_0)
  Rv_t = (Rv_{t-1} + C_{t-1}) * v_t (u-run then v-run to t;   Rv_1 = C_0*v_1)
  out[:, 0:26]  = ln(C_3 + e_4) + Sb
  out[:, 26:52] = ln(Rv_1 + Rv_2 + Rv_3 + Rv_4) + Sb
Linear-space bf16 is safe: |log path scores| <= ~30.

Sharding: pure data parallel, batch 16 -> 2 per core across 8 cores.
Host prep: slice channels 0..27 (all the automata read), zero-pad time by
2, cast to bf16 -> [2, 4100, 28] per core.  Output bf16 -> f32 on host.

Engine split (cost model: ~ns = free_size * cycle_t / speedup):
  DVE  : chain muls/adds (bf16 2x tt, 4x ts)
  Pool : XD subtract + Sb window-reduce + Sb broadcast-adds + overflow
         chain steps as fused scalar_tensor_tensor (0.6 eff)
  ACT  : one shared exp per batch (u/v are overlapping column views),
         Ln of G1 (SBUF) and G2 (PSUM)
  PE   : G2 = Rv_1+..+Rv_4 as identity-matmul accumulation into PSUM
"""

from contextlib import ExitStack

import numpy as np

import concourse.bacc as bacc
import concourse.bass as bass
import concourse.mybir as mybir
import concourse.tile as tile
from concourse.bass_utils import run_bass_kernel_spmd

F32 = mybir.dt.float32
BF16 = mybir.dt.bfloat16
A = mybir.AluOpType
AF = mybir.ActivationFunctionType

B_FULL, T, C = 16, 4096, 128
KTAPS = 5
PAD = 2
TP = T + 2 * PAD
CH = 28          # channels shipped: blank + labels 1..27
NK = 26          # lexicon entries per type
NCOL = 52        # output channels
N_CORES = 8
B_CORE = B_FULL // N_CORES
WPP = 32         # windows per partition (128 * 32 = 4096)
ROWS = WPP + KTAPS - 1


def _mkap(ap, dims, extra_offset=0):
    """Manual AP on the same tensor: keep partition dim, replace free dims."""
    part = ap.ap[0]
    return bass.AP(ap.tensor, ap.offset + extra_offset,
                   [list(part)] + [list(d) for d in dims])


# Engine assignment knobs (tuned against the TimelineSim cost model).
# Walrus codegen limits: Pool takes tensor_tensor / plain tensor_scalar
# only (no scalar_tensor_tensor, no PSUM access); Ln/Exp are ACT-only.
CFG = dict(
    e_ts_eng=("v", "a", "a", "p"),     # engine for the +1 of e_1..e_4
    e_tt_eng=("v", "v", "v", "v"),     # engine for the *u_j of e_1..e_4
    a_add_eng=("v", "v", "v"),         # A_2..A_4 adds
    c_add_eng=("v", "v", "v"),         # C_1..C_3 adds
    sb1_add_eng=("p", "p"),            # per-batch engine for OUT1 += Sb
    sb2_add_eng=("p", "p"),            # per-batch engine for OUT2 += Sb
    xd_eng=("v", "p"),  # per-batch: b0 on DVE (idle during fill), b1 Pool
)


def _build_core_kernel(nc, cfg=CFG, b_core=B_CORE):
    x = nc.declare_dram_parameter("x", [b_core, TP, CH], BF16, isOutput=False)
    ident = nc.declare_dram_parameter("ident", [128, 128], BF16, isOutput=False)
    # type-major output: y[b, type, w, c]; host concatenates types on last axis
    y = nc.declare_dram_parameter("y", [b_core, 2, T, NK], BF16, isOutput=True)

    with ExitStack() as ctx:
        tc = ctx.enter_context(tile.TileContext(nc))
        const = ctx.enter_context(tc.tile_pool(name="const", bufs=1))
        pool = ctx.enter_context(tc.tile_pool(name="main", bufs=2))
        rot = ctx.enter_context(tc.tile_pool(name="rot", bufs=4))
        psum = ctx.enter_context(tc.tile_pool(name="ps", bufs=2, space="PSUM"))

        v = nc.vector
        g = nc.gpsimd
        s = nc.scalar
        pe = nc.tensor

        # ---- stage 1: input DMA for both batches first, ident last ----
        X3s = []
        for b in range(b_core):
            X3 = pool.tile([128, ROWS, CH], BF16, tag="X3", name=f"X3_{b}")
            nc.sync.dma_start(
                out=X3[:],
                in_=bass.AP(x, b * TP * CH,
                            [[WPP * CH, 128], [CH, ROWS], [1, CH]]))
            X3s.append(X3)
        ID = const.tile([128, 128], BF16, tag="ID")
        nc.sync.dma_start(out=ID[:], in_=ident.ap())

        # ---- stage 2: XD + exp per batch, interleaved for earliest start ----
        XDs, ESbs, EXs = [], [], []
        for b in range(b_core):
            X3 = X3s[b]
            XD = pool.tile([128, ROWS, CH], BF16, tag="XD", name=f"XD_{b}")
            eng = g if cfg["xd_eng"][b] == "p" else v
            eng.tensor_tensor(
                XD[:, :, 0:CH - 1], X3[:, :, 1:CH],
                X3[:, :, 0:1].broadcast_to([128, ROWS, CH - 1]), A.subtract)
            XDs.append(XD)
            EX = pool.tile([128, ROWS, CH], BF16, tag="EX", name=f"EX_{b}")
            s.activation(EX[:, :, 0:CH - 1], XD[:, :, 0:CH - 1], AF.Exp)
            EXs.append(EX)
        for b in range(b_core):
            Sb = pool.tile([128, WPP], F32, tag="Sb", name=f"Sb_{b}")
            v.tensor_reduce(
                Sb[:], _mkap(X3s[b][:], [[CH, WPP], [CH, KTAPS]]),
                mybir.AxisListType.X, A.add)
            ESbs.append(Sb)

        # window views into EX: u_t cols 0:26 (labels 1..26), v_t cols 1:27
        def U(b, t):
            return EXs[b][:, t:t + WPP, 0:NK]

        def V(b, t):
            return EXs[b][:, t:t + WPP, 1:NK + 1]

        def pt(tag, b):
            return rot.tile([128, WPP, NK], BF16, tag=tag, name=f"{tag}_{b}")

        # ---- stage 3: chains, batch-interleaved step by step ----
        e = [[None] * KTAPS for _ in range(b_core)]   # e_1..e_4 tiles
        Cp = [[None] * KTAPS for _ in range(b_core)]  # C_1..C_3 tiles
        Rv = [[None] * KTAPS for _ in range(b_core)]
        G1ps = [None] * b_core
        G2ps = [None] * b_core
        HNK = NK // 2
        for b in range(b_core):
            G1ps[b] = [psum.tile([128, WPP, HNK], F32, tag=f"g1p{h}",
                                 name=f"g1p{h}_{b}") for h in range(2)]
            G2ps[b] = [psum.tile([128, WPP, HNK], F32, tag=f"g2p{h}",
                                 name=f"g2p{h}_{b}") for h in range(2)]

        def e_step(b, j):
            prev = U(b, 0) if j == 1 else e[b][j - 1][:]
            ej = pt("e", b)
            tmp = pt("t1", b)
            ts_eng = cfg["e_ts_eng"][j - 1]
            if ts_eng == "a":
                s.activation(tmp[:], prev, AF.Copy, bias=1.0)
            elif ts_eng == "p":
                g.tensor_scalar_add(tmp[:], prev, 1.0)
            else:
                v.tensor_scalar_add(tmp[:], prev, 1.0)
            if cfg["e_tt_eng"][j - 1] == "p":
                g.tensor_tensor(ej[:], tmp[:], U(b, j), A.mult)
            else:
                v.tensor_tensor(ej[:], tmp[:], U(b, j), A.mult)
            e[b][j] = ej

        def c_step(b, j):
            prev = U(b, 0) if j == 1 else Cp[b][j - 1][:]
            Cj = pt("C", b)
            eng = g if cfg["c_add_eng"][j - 1] == "p" else v
            eng.tensor_tensor(Cj[:], prev, e[b][j][:], A.add)
            Cp[b][j] = Cj

        def rv_step(b, k):
            if k == 1:
                Rv1 = pt("Rv", b)
                v.tensor_tensor(Rv1[:], U(b, 0), V(b, 1), A.mult)
                Rv[b][1] = Rv1
                return
            Ak = pt("Ak", b)
            eng = g if cfg["a_add_eng"][k - 2] == "p" else v
            eng.tensor_tensor(Ak[:], Rv[b][k - 1][:], Cp[b][k - 1][:], A.add)
            Rvk = pt("Rv", b)
            v.tensor_tensor(Rvk[:], Ak[:], V(b, k), A.mult)
            Rv[b][k] = Rvk

        def pe_accum(b, k):
            for h in range(2):
                pe.matmul(G2ps[b][h][:], lhsT=ID[:],
                          rhs=Rv[b][k][:, :, h * HNK:(h + 1) * HNK],
                          start=(k == 1), stop=(k == KTAPS - 1))

        def pe_g1(b):
            # G1 = C3 + e4 accumulated on the idle PE
            for h in range(2):
                sl = slice(h * HNK, (h + 1) * HNK)
                pe.matmul(G1ps[b][h][:], lhsT=ID[:],
                          rhs=Cp[b][3][:, :, sl], start=True, stop=False)
                pe.matmul(G1ps[b][h][:], lhsT=ID[:],
                          rhs=e[b][4][:, :, sl], start=False, stop=True)

        def scale_ln_dma(b, which):
            """which: 0 = G1 path, 1 = G2 path. Ln straight from the PSUM
            halves into bf16 SBUF, add Sb post-ln, DMA out."""
            ps = G1ps[b] if which == 0 else G2ps[b]
            eng = (cfg["sb1_add_eng"] if which == 0 else
                   cfg["sb2_add_eng"])[b]
            sb_ap = _mkap(ESbs[b][:], [[1, WPP], [0, NK]])
            OUT = pool.tile([128, WPP, NK], BF16, tag=f"OUT{which}",
                            name=f"O{which}_{b}")
            for h in range(2):
                s.activation(OUT[:, :, h * HNK:(h + 1) * HNK], ps[h][:],
                             AF.Ln)
            eng_obj = g if eng == "p" else v
            eng_obj.tensor_tensor(OUT[:], OUT[:], sb_ap, A.add)
            nc.sync.dma_start(
                out=bass.AP(y, (b * 2 + which) * T * NK,
                            [[WPP * NK, 128], [NK, WPP], [1, NK]]),
                in_=OUT[:])

        for b in range(b_core):
            e_step(b, 1)
        for b in range(b_core):
            rv_step(b, 1)
            pe_accum(b, 1)
        for b in range(b_core):
            c_step(b, 1)
            e_step(b, 2)
        for b in range(b_core):
            rv_step(b, 2)
            pe_accum(b, 2)
        for b in range(b_core):
            c_step(b, 2)
            e_step(b, 3)
        for b in range(b_core):
            c_step(b, 3)
            e_step(b, 4)
        for b in range(b_core):
            pe_g1(b)
        for b in range(b_core):
            rv_step(b, 3)
            pe_accum(b, 3)
        scale_ln_dma(0, 0)         # type-1 ships while Rv4/G2 still run
        scale_ln_dma(1, 0)
        for b in range(b_core):
            rv_step(b, 4)
            pe_accum(b, 4)
        scale_ln_dma(0, 1)
        scale_ln_dma(1, 1)
    return nc


_NC_CACHE = {}


def _get_nc():
    if "nc" not in _NC_CACHE:
        nc = bacc.Bacc()
        _build_core_kernel(nc)
        nc.compile()
        _NC_CACHE["nc"] = nc
    return _NC_CACHE["nc"]


_BF16_NP = mybir.dt.np(BF16)


def _prep_shard(x_shard):
    """[B_CORE, T, C] f32 -> zero-padded, channel-sliced bf16 [B_CORE, TP, CH]."""
    out = np.zeros((x_shard.shape[0], TP, CH), _BF16_NP)
    out[:, PAD:PAD + T, :] = x_shard[:, :, 0:CH].astype(_BF16_NP)
    return out


def _run(x, trace=False, **kw):
    x = np.asarray(x, dtype=np.float32)
    assert x.shape == (B_FULL, T, C), x.shape
    nc = _get_nc()
    ident = np.eye(128, dtype=_BF16_NP)
    in_maps = [{"x": _prep_shard(x[i * B_CORE:(i + 1) * B_CORE]),
                "ident": ident}
               for i in range(N_CORES)]
    res = run_bass_kernel_spmd(nc, in_maps, list(range(N_CORES)),
                               trace=trace, **kw)
    # y[b, type, w, c] -> out[b, w, type*26 + c]
    ys = np.concatenate([res.results[i]["y"] for i in range(N_CORES)], axis=0)
    out = np.concatenate([ys[:, 0], ys[:, 1]], axis=-1)
    return np.ascontiguousarray(out.astype(np.float32)), res


def kernel(x):
    out, _ = _run(x, trace=False)
    return out


# revision 34
# speedup vs baseline: 1.0362x; 1.0362x over previous
"""Trainium2 Bass kernel for nn_ConvTransduce1D (self-contained).

Computes, for x [16, 4096, 128] fp32, the CTC-style automaton forward scores
out [16, 4096, 52] of 52 tiny lexicon automata (26 single-token [c], 26
two-token [c, c+1], c = 1..26, blank = 0) over sliding windows of K=5 frames
(stride 1, pad 2).

Closed form (validated against the jax reference):
  For window w, padded frames give u_t = exp(x[w+t, c] - x[w+t, 0]) and
  v_t = exp(x[w+t, c+1] - x[w+t, 0]), t = 0..4; Sb = sum_t x[w+t, 0].
  e_t  = u_t * (1 + e_{t-1})        (runs of c ending at t;   e_0 = u_0)
  C_t  = C_{t-1} + e_t              (prefix sums;             C_0 = e_0)
  Rv_t = (Rv_{t-1} + C_{t-1}) * v_t (u-run then v-run to t;   Rv_1 = C_0*v_1)
  out[:, 0:26]  = ln(C_3 + e_4) + Sb
  out[:, 26:52] = ln(Rv_1 + Rv_2 + Rv_3 + Rv_4) + Sb
Linear-space bf16 is safe: |log path scores| <= ~30.

Sharding: pure data parallel, batch 16 -> 2 per core across 8 cores.
Host prep: slice channels 0..27 (all the automata read), zero-pad time by
2, cast to bf16 -> [2, 4100, 28] per core.  Output bf16 -> f32 on host.

Engine split (cost model: ~ns = free_size * cycle_t / speedup):
  DVE  : chain muls/adds (bf16 2x tt, 4x ts)
  Pool : XD subtract + Sb window-reduce + Sb broadcast-adds + overflow
         chain steps as fused scalar_tensor_tensor (0.6 eff)
  ACT  : one shared exp per batch (u/v are overlapping column views),
         Ln of G1 (SBUF) and G2 (PSUM)
  PE   : G2 = Rv_1+..+Rv_4 as identity-matmul accumulation into PSUM
"""

from contextlib import ExitStack

import numpy as np

import concourse.bacc as bacc
import concourse.bass as bass
import concourse.mybir as mybir
import concourse.tile as tile
from concourse.bass_utils import run_bass_kernel_spmd

F32 = mybir.dt.float32
BF16 = mybir.dt.bfloat16
A = mybir.AluOpType
AF = mybir.ActivationFunctionType

B_FULL, T, C = 16, 4096, 128
KTAPS = 5
PAD = 2
TP = T + 2 * PAD
CH = 28          # channels shipped: blank + labels 1..27
NK = 26          # lexicon entries per type
NCOL = 52        # output channels
N_CORES = 8
B_CORE = B_FULL // N_CORES
WPP = 32         # windows per partition (128 * 32 = 4096)
ROWS = WPP + KTAPS - 1


def _mkap(ap, dims, extra_offset=0):
    """Manual AP on the same tensor: keep partition dim, replace free dims."""
    part = ap.ap[0]
    return bass.AP(ap.tensor, ap.offset + extra_offset,
                   [list(part)] + [list(d) for d in dims])


# Engine assignment knobs (tuned against the TimelineSim cost model).
# Walrus codegen limits: Pool takes tensor_tensor / plain tensor_scalar
# only (no scalar_tensor_tensor, no PSUM access); Ln/Exp are ACT-only.
CFG = dict(
    e_ts_eng=("v", "a", "a", "v"),     # engine for the +1 of e_1..e_4
    e_tt_eng=("v", "v", "v", "v"),     # engine for the *u_j of e_1..e_4
    a_add_eng=("v", "v", "v"),         # A_2..A_4 adds
    c_add_eng=("v", "v", "v"),         # C_1..C_3 adds
    sb1_add_eng=("p", "p"),            # per-batch engine for OUT1 += Sb
    sb2_add_eng=("v", "v"),            # per-batch engine for OUT2 += Sb
    xd_eng=("v", "p"),  # per-batch: b0 on DVE (idle during fill), b1 Pool
)


def _build_core_kernel(nc, cfg=CFG, b_core=B_CORE):
    x = nc.declare_dram_parameter("x", [b_core, TP, CH], BF16, isOutput=False)
    ident = nc.declare_dram_parameter("ident", [128, 128], BF16, isOutput=False)
    # type-major output: y[b, type, w, c]; host concatenates types on last axis
    y = nc.declare_dram_parameter("y", [b_core, 2, T, NK], BF16, isOutput=True)

    with ExitStack() as ctx:
        tc = ctx.enter_context(tile.TileContext(nc))
        const = ctx.enter_context(tc.tile_pool(name="const", bufs=1))
        pool = ctx.enter_context(tc.tile_pool(name="main", bufs=2))
        rot = ctx.enter_context(tc.tile_pool(name="rot", bufs=4))
        psum = ctx.enter_context(tc.tile_pool(name="ps", bufs=2, space="PSUM"))

        v = nc.vector
        g = nc.gpsimd
        s = nc.scalar
        pe = nc.tensor

        # ---- stage 1: input DMA for both batches first, ident last ----
        X3s = []
        for b in range(b_core):
            X3 = pool.tile([128, ROWS, CH], BF16, tag="X3", name=f"X3_{b}")
            nc.sync.dma_start(
                out=X3[:],
                in_=bass.AP(x, b * TP * CH,
                            [[WPP * CH, 128], [CH, ROWS], [1, CH]]))
            X3s.append(X3)
        ID = const.tile([128, 128], BF16, tag="ID")
        nc.sync.dma_start(out=ID[:], in_=ident.ap())

        # ---- stage 2: XD + exp per batch, interleaved for earliest start ----
        XDs, ESbs, EXs = [], [], []
        for b in range(b_core):
            X3 = X3s[b]
            XD = pool.tile([128, ROWS, CH], BF16, tag="XD", name=f"XD_{b}")
            eng = g if cfg["xd_eng"][b] == "p" else v
            eng.tensor_tensor(
                XD[:, :, 0:CH - 1], X3[:, :, 1:CH],
                X3[:, :, 0:1].broadcast_to([128, ROWS, CH - 1]), A.subtract)
            XDs.append(XD)
            EX = pool.tile([128, ROWS, CH], BF16, tag="EX", name=f"EX_{b}")
            s.activation(EX[:, :, 0:CH - 1], XD[:, :, 0:CH - 1], AF.Exp)
            EXs.append(EX)
        for b in range(b_core):
            Sb = pool.tile([128, WPP], F32, tag="Sb", name=f"Sb_{b}")
            v.tensor_reduce(
                Sb[:], _mkap(X3s[b][:], [[CH, WPP], [CH, KTAPS]]),
                mybir.AxisListType.X, A.add)
            ESbs.append(Sb)

        # window views into EX: u_t cols 0:26 (labels 1..26), v_t cols 1:27
        def U(b, t):
            return EXs[b][:, t:t + WPP, 0:NK]

        def V(b, t):
            return EXs[b][:, t:t + WPP, 1:NK + 1]

        def pt(tag, b):
            return rot.tile([128, WPP, NK], BF16, tag=tag, name=f"{tag}_{b}")

        # ---- stage 3: chains, batch-interleaved step by step ----
        e = [[None] * KTAPS for _ in range(b_core)]   # e_1..e_4 tiles
        Cp = [[None] * KTAPS for _ in range(b_core)]  # C_1..C_3 tiles
        Rv = [[None] * KTAPS for _ in range(b_core)]
        G1ps = [None] * b_core
        G2ps = [None] * b_core
        HNK = NK // 2
        for b in range(b_core):
            G1ps[b] = [psum.tile([128, WPP, HNK], F32, tag=f"g1p{h}",
                                 name=f"g1p{h}_{b}") for h in range(2)]
            G2ps[b] = [psum.tile([128, WPP, HNK], F32, tag=f"g2p{h}",
                                 name=f"g2p{h}_{b}") for h in range(2)]

        def e_step(b, j):
            prev = U(b, 0) if j == 1 else e[b][j - 1][:]
            ej = pt("e", b)
            tmp = pt("t1", b)
            ts_eng = cfg["e_ts_eng"][j - 1]
            if ts_eng == "a":
                s.activation(tmp[:], prev, AF.Copy, bias=1.0)
            elif ts_eng == "p":
                g.tensor_scalar_add(tmp[:], prev, 1.0)
            else:
                v.tensor_scalar_add(tmp[:], prev, 1.0)
            if cfg["e_tt_eng"][j - 1] == "p":
                g.tensor_tensor(ej[:], tmp[:], U(b, j), A.mult)
            else:
                v.tensor_tensor(ej[:], tmp[:], U(b, j), A.mult)
            e[b][j] = ej

        def c_step(b, j):
            prev = U(b, 0) if j == 1 else Cp[b][j - 1][:]
            Cj = pt("C", b)
            eng = g if cfg["c_add_eng"][j - 1] == "p" else v
            eng.tensor_tensor(Cj[:], prev, e[b][j][:], A.add)
            Cp[b][j] = Cj

        def rv_step(b, k):
            if k == 1:
                Rv1 = pt("Rv", b)
                v.tensor_tensor(Rv1[:], U(b, 0), V(b, 1), A.mult)
                Rv[b][1] = Rv1
                return
            Ak = pt("Ak", b)
            eng = g if cfg["a_add_eng"][k - 2] == "p" else v
            eng.tensor_tensor(Ak[:], Rv[b][k - 1][:], Cp[b][k - 1][:], A.add)
            Rvk = pt("Rv", b)
            v.tensor_tensor(Rvk[:], Ak[:], V(b, k), A.mult)
            Rv[b][k] = Rvk

        def pe_accum(b, k):
            for h in range(2):
                pe.matmul(G2ps[b][h][:], lhsT=ID[:],
                          rhs=Rv[b][k][:, :, h * HNK:(h + 1) * HNK],
                          start=(k == 1), stop=(k == KTAPS - 1))

        def pe_g1(b):
            # G1 = C3 + e4 accumulated on the idle PE
            for h in range(2):
                sl = slice(h * HNK, (h + 1) * HNK)
                pe.matmul(G1ps[b][h][:], lhsT=ID[:],
                          rhs=Cp[b][3][:, :, sl], start=True, stop=False)
                pe.matmul(G1ps[b][h][:], lhsT=ID[:],
                          rhs=e[b][4][:, :, sl], start=False, stop=True)

        def scale_ln_dma(b, which):
            """which: 0 = G1 path, 1 = G2 path. Ln straight from the PSUM
            halves into bf16 SBUF, add Sb post-ln, DMA out."""
            ps = G1ps[b] if which == 0 else G2ps[b]
            eng = (cfg["sb1_add_eng"] if which == 0 else
                   cfg["sb2_add_eng"])[b]
            sb_ap = _mkap(ESbs[b][:], [[1, WPP], [0, NK]])
            OUT = pool.tile([128, WPP, NK], BF16, tag=f"OUT{which}",
                            name=f"O{which}_{b}")
            for h in range(2):
                s.activation(OUT[:, :, h * HNK:(h + 1) * HNK], ps[h][:],
                             AF.Ln)
            eng_obj = g if eng == "p" else v
            eng_obj.tensor_tensor(OUT[:], OUT[:], sb_ap, A.add)
            nc.sync.dma_start(
                out=bass.AP(y, (b * 2 + which) * T * NK,
                            [[WPP * NK, 128], [NK, WPP], [1, NK]]),
                in_=OUT[:])

        for b in range(b_core):
            e_step(b, 1)
        for b in range(b_core):
            rv_step(b, 1)
            pe_accum(b, 1)
        for b in range(b_core):
            c_step(b, 1)
            e_step(b, 2)
        for b in range(b_core):
            rv_step(b, 2)
            pe_accum(b, 2)
        for b in range(b_core):
            c_step(b, 2)
            e_step(b, 3)
        for b in range(b_core):
            c_step(b, 3)
            e_step(b, 4)
        for b in range(b_core):
            pe_g1(b)
        for b in range(b_core):
            rv_step(b, 3)
            pe_accum(b, 3)
        scale_ln_dma(0, 0)         # type-1 ships while Rv4/G2 still run
        scale_ln_dma(1, 0)
        for b in range(b_core):
            rv_step(b, 4)
            pe_accum(b, 4)
        scale_ln_dma(0, 1)
        scale_ln_dma(1, 1)
    return nc


_NC_CACHE = {}


def _get_nc():
    if "nc" not in _NC_CACHE:
        nc = bacc.Bacc()
        _build_core_kernel(nc)
        nc.compile()
        _NC_CACHE["nc"] = nc
    return _NC_CACHE["nc"]


_BF16_NP = mybir.dt.np(BF16)


def _prep_shard(x_shard):
    """[B_CORE, T, C] f32 -> zero-padded, channel-sliced bf16 [B_CORE, TP, CH]."""
    out = np.zeros((x_shard.shape[0], TP, CH), _BF16_NP)
    out[:, PAD:PAD + T, :] = x_shard[:, :, 0:CH].astype(_BF16_NP)
    return out


def _run(x, trace=False, **kw):
    x = np.asarray(x, dtype=np.float32)
    assert x.shape == (B_FULL, T, C), x.shape
    nc = _get_nc()
    ident = np.eye(128, dtype=_BF16_NP)
    in_maps = [{"x": _prep_shard(x[i * B_CORE:(i + 1) * B_CORE]),
                "ident": ident}
               for i in range(N_CORES)]
    res = run_bass_kernel_spmd(nc, in_maps, list(range(N_CORES)),
                               trace=trace, **kw)
    # y[b, type, w, c] -> out[b, w, type*26 + c]
    ys = np.concatenate([res.results[i]["y"] for i in range(N_CORES)], axis=0)
    out = np.concatenate([ys[:, 0], ys[:, 1]], axis=-1)
    return np.ascontiguousarray(out.astype(np.float32)), res


def kernel(x):
    out, _ = _run(x, trace=False)
    return out


# revision 36
# speedup vs baseline: 1.0460x; 1.0095x over previous
"""Trainium2 Bass kernel for nn_ConvTransduce1D (self-contained).

Computes, for x [16, 4096, 128] fp32, the CTC-style automaton forward scores
out [16, 4096, 52] of 52 tiny lexicon automata (26 single-token [c], 26
two-token [c, c+1], c = 1..26, blank = 0) over sliding windows of K=5 frames
(stride 1, pad 2).

Closed form (validated against the jax reference):
  For window w, padded frames give u_t = exp(x[w+t, c] - x[w+t, 0]) and
  v_t = exp(x[w+t, c+1] - x[w+t, 0]), t = 0..4; Sb = sum_t x[w+t, 0].
  e_t  = u_t * (1 + e_{t-1})        (runs of c ending at t;   e_0 = u_0)
  C_t  = C_{t-1} + e_t              (prefix sums;             C_0 = e_0)
  Rv_t = (Rv_{t-1} + C_{t-1}) * v_t (u-run then v-run to t;   Rv_1 = C_0*v_1)
  out[:, 0:26]  = ln(C_3 + e_4) + Sb
  out[:, 26:52] = ln(Rv_1 + Rv_2 + Rv_3 + Rv_4) + Sb
Linear-space bf16 is safe: |log path scores| <= ~30.

Sharding: pure data parallel, batch 16 -> 2 per core across 8 cores.
Host prep: slice channels 0..27 (all the automata read), zero-pad time by
2, cast to bf16 -> [2, 4100, 28] per core.  Output bf16 -> f32 on host.

Engine split (cost model: ~ns = free_size * cycle_t / speedup):
  DVE  : chain muls/adds (bf16 2x tt, 4x ts)
  Pool : XD subtract + Sb window-reduce + Sb broadcast-adds + overflow
         chain steps as fused scalar_tensor_tensor (0.6 eff)
  ACT  : one shared exp per batch (u/v are overlapping column views),
         Ln of G1 (SBUF) and G2 (PSUM)
  PE   : G2 = Rv_1+..+Rv_4 as identity-matmul accumulation into PSUM
"""

from contextlib import ExitStack

import numpy as np

import concourse.bacc as bacc
import concourse.bass as bass
import concourse.mybir as mybir
import concourse.tile as tile
from concourse.bass_utils import run_bass_kernel_spmd

F32 = mybir.dt.float32
BF16 = mybir.dt.bfloat16
A = mybir.AluOpType
AF = mybir.ActivationFunctionType

B_FULL, T, C = 16, 4096, 128
KTAPS = 5
PAD = 2
TP = T + 2 * PAD
CH = 28          # channels shipped: blank + labels 1..27
NK = 26          # lexicon entries per type
NCOL = 52        # output channels
N_CORES = 8
B_CORE = B_FULL // N_CORES
WPP = 32         # windows per partition (128 * 32 = 4096)
ROWS = WPP + KTAPS - 1


def _mkap(ap, dims, extra_offset=0):
    """Manual AP on the same tensor: keep partition dim, replace free dims."""
    part = ap.ap[0]
    return bass.AP(ap.tensor, ap.offset + extra_offset,
                   [list(part)] + [list(d) for d in dims])


# Engine assignment knobs (tuned against the TimelineSim cost model).
# Walrus codegen limits: Pool takes tensor_tensor / plain tensor_scalar
# only (no scalar_tensor_tensor, no PSUM access); Ln/Exp are ACT-only.
CFG = dict(
    e_ts_eng=("v", "a", "a", "v"),     # engine for the +1 of e_1..e_4
    e_tt_eng=("v", "v", "v", "v"),     # engine for the *u_j of e_1..e_4
    a_add_eng=("v", "v", "v"),         # A_2..A_4 adds
    c_add_eng=("v", "p", "v"),         # C_1..C_3 adds
    sb1_add_eng=("p", "p"),            # per-batch engine for OUT1 += Sb
    sb2_add_eng=("v", "v"),            # per-batch engine for OUT2 += Sb
    xd_eng=("v", "v"),  # per-batch XD engine (DVE is idle during fill)
)


def _build_core_kernel(nc, cfg=CFG, b_core=B_CORE):
    x = nc.declare_dram_parameter("x", [b_core, TP, CH], BF16, isOutput=False)
    ident = nc.declare_dram_parameter("ident", [128, 128], BF16, isOutput=False)
    # type-major output: y[b, type, w, c]; host concatenates types on last axis
    y = nc.declare_dram_parameter("y", [b_core, 2, T, NK], BF16, isOutput=True)

    with ExitStack() as ctx:
        tc = ctx.enter_context(tile.TileContext(nc))
        const = ctx.enter_context(tc.tile_pool(name="const", bufs=1))
        pool = ctx.enter_context(tc.tile_pool(name="main", bufs=2))
        rot = ctx.enter_context(tc.tile_pool(name="rot", bufs=4))
        psum = ctx.enter_context(tc.tile_pool(name="ps", bufs=2, space="PSUM"))

        v = nc.vector
        g = nc.gpsimd
        s = nc.scalar
        pe = nc.tensor

        # ---- stage 1: input DMA for both batches first, ident last ----
        X3s = []
        for b in range(b_core):
            X3 = pool.tile([128, ROWS, CH], BF16, tag="X3", name=f"X3_{b}")
            nc.sync.dma_start(
                out=X3[:],
                in_=bass.AP(x, b * TP * CH,
                            [[WPP * CH, 128], [CH, ROWS], [1, CH]]))
            X3s.append(X3)
        ID = const.tile([128, 128], BF16, tag="ID")
        nc.sync.dma_start(out=ID[:], in_=ident.ap())

        # ---- stage 2: XD + exp per batch, interleaved for earliest start ----
        XDs, ESbs, EXs = [], [], []
        for b in range(b_core):
            X3 = X3s[b]
            XD = pool.tile([128, ROWS, CH], BF16, tag="XD", name=f"XD_{b}")
            eng = g if cfg["xd_eng"][b] == "p" else v
            eng.tensor_tensor(
                XD[:, :, 0:CH - 1], X3[:, :, 1:CH],
                X3[:, :, 0:1].broadcast_to([128, ROWS, CH - 1]), A.subtract)
            XDs.append(XD)
            EX = pool.tile([128, ROWS, CH], BF16, tag="EX", name=f"EX_{b}")
            s.activation(EX[:, :, 0:CH - 1], XD[:, :, 0:CH - 1], AF.Exp)
            EXs.append(EX)
        for b in range(b_core):
            Sb = pool.tile([128, WPP], F32, tag="Sb", name=f"Sb_{b}")
            v.tensor_reduce(
                Sb[:], _mkap(X3s[b][:], [[CH, WPP], [CH, KTAPS]]),
                mybir.AxisListType.X, A.add)
            ESbs.append(Sb)

        # window views into EX: u_t cols 0:26 (labels 1..26), v_t cols 1:27
        def U(b, t):
            return EXs[b][:, t:t + WPP, 0:NK]

        def V(b, t):
            return EXs[b][:, t:t + WPP, 1:NK + 1]

        def pt(tag, b):
            return rot.tile([128, WPP, NK], BF16, tag=tag, name=f"{tag}_{b}")

        # ---- stage 3: chains, batch-interleaved step by step ----
        e = [[None] * KTAPS for _ in range(b_core)]   # e_1..e_4 tiles
        Cp = [[None] * KTAPS for _ in range(b_core)]  # C_1..C_3 tiles
        Rv = [[None] * KTAPS for _ in range(b_core)]
        G1ps = [None] * b_core
        G2ps = [None] * b_core
        HNK = NK // 2
        for b in range(b_core):
            G1ps[b] = [psum.tile([128, WPP, HNK], F32, tag=f"g1p{h}",
                                 name=f"g1p{h}_{b}") for h in range(2)]
            G2ps[b] = [psum.tile([128, WPP, HNK], F32, tag=f"g2p{h}",
                                 name=f"g2p{h}_{b}") for h in range(2)]

        def e_step(b, j):
            prev = U(b, 0) if j == 1 else e[b][j - 1][:]
            ej = pt("e", b)
            tmp = pt("t1", b)
            ts_eng = cfg["e_ts_eng"][j - 1]
            if ts_eng == "a":
                s.activation(tmp[:], prev, AF.Copy, bias=1.0)
            elif ts_eng == "p":
                g.tensor_scalar_add(tmp[:], prev, 1.0)
            else:
                v.tensor_scalar_add(tmp[:], prev, 1.0)
            if cfg["e_tt_eng"][j - 1] == "p":
                g.tensor_tensor(ej[:], tmp[:], U(b, j), A.mult)
            else:
                v.tensor_tensor(ej[:], tmp[:], U(b, j), A.mult)
            e[b][j] = ej

        def c_step(b, j):
            prev = U(b, 0) if j == 1 else Cp[b][j - 1][:]
            Cj = pt("C", b)
            eng = g if cfg["c_add_eng"][j - 1] == "p" else v
            eng.tensor_tensor(Cj[:], prev, e[b][j][:], A.add)
            Cp[b][j] = Cj

        def rv_step(b, k):
            if k == 1:
                Rv1 = pt("Rv", b)
                v.tensor_tensor(Rv1[:], U(b, 0), V(b, 1), A.mult)
                Rv[b][1] = Rv1
                return
            Ak = pt("Ak", b)
            eng = g if cfg["a_add_eng"][k - 2] == "p" else v
            eng.tensor_tensor(Ak[:], Rv[b][k - 1][:], Cp[b][k - 1][:], A.add)
            Rvk = pt("Rv", b)
            v.tensor_tensor(Rvk[:], Ak[:], V(b, k), A.mult)
            Rv[b][k] = Rvk

        def pe_accum(b, k):
            for h in range(2):
                pe.matmul(G2ps[b][h][:], lhsT=ID[:],
                          rhs=Rv[b][k][:, :, h * HNK:(h + 1) * HNK],
                          start=(k == 1), stop=(k == KTAPS - 1))

        def pe_g1(b):
            # G1 = C3 + e4 accumulated on the idle PE
            for h in range(2):
                sl = slice(h * HNK, (h + 1) * HNK)
                pe.matmul(G1ps[b][h][:], lhsT=ID[:],
                          rhs=Cp[b][3][:, :, sl], start=True, stop=False)
                pe.matmul(G1ps[b][h][:], lhsT=ID[:],
                          rhs=e[b][4][:, :, sl], start=False, stop=True)

        def scale_ln_dma(b, which):
            """which: 0 = G1 path, 1 = G2 path. Ln straight from the PSUM
            halves into bf16 SBUF, add Sb post-ln, DMA out."""
            ps = G1ps[b] if which == 0 else G2ps[b]
            eng = (cfg["sb1_add_eng"] if which == 0 else
                   cfg["sb2_add_eng"])[b]
            sb_ap = _mkap(ESbs[b][:], [[1, WPP], [0, NK]])
            OUT = pool.tile([128, WPP, NK], BF16, tag=f"OUT{which}",
                            name=f"O{which}_{b}")
            for h in range(2):
                s.activation(OUT[:, :, h * HNK:(h + 1) * HNK], ps[h][:],
                             AF.Ln)
            eng_obj = g if eng == "p" else v
            eng_obj.tensor_tensor(OUT[:], OUT[:], sb_ap, A.add)
            nc.sync.dma_start(
                out=bass.AP(y, (b * 2 + which) * T * NK,
                            [[WPP * NK, 128], [NK, WPP], [1, NK]]),
                in_=OUT[:])

        for b in range(b_core):
            e_step(b, 1)
        for b in range(b_core):
            rv_step(b, 1)
            pe_accum(b, 1)
        for b in range(b_core):
            c_step(b, 1)
            e_step(b, 2)
        for b in range(b_core):
            rv_step(b, 2)
            pe_accum(b, 2)
        for b in range(b_core):
            c_step(b, 2)
            e_step(b, 3)
        for b in range(b_core):
            c_step(b, 3)
            e_step(b, 4)
        for b in range(b_core):
            pe_g1(b)
        for b in range(b_core):
            rv_step(b, 3)
            pe_accum(b, 3)
        scale_ln_dma(0, 0)         # type-1 ships while Rv4/G2 still run
        scale_ln_dma(1, 0)
        for b in range(b_core):
            rv_step(b, 4)
            pe_accum(b, 4)
        scale_ln_dma(0, 1)
        scale_ln_dma(1, 1)
    return nc


_NC_CACHE = {}


def _get_nc():
    if "nc" not in _NC_CACHE:
        nc = bacc.Bacc()
        _build_core_kernel(nc)
        nc.compile()
        _NC_CACHE["nc"] = nc
    return _NC_CACHE["nc"]


_BF16_NP = mybir.dt.np(BF16)


def _prep_shard(x_shard):
    """[B_CORE, T, C] f32 -> zero-padded, channel-sliced bf16 [B_CORE, TP, CH]."""
    out = np.zeros((x_shard.shape[0], TP, CH), _BF16_NP)
    out[:, PAD:PAD + T, :] = x_shard[:, :, 0:CH].astype(_BF16_NP)
    return out


def _run(x, trace=False, **kw):
    x = np.asarray(x, dtype=np.float32)
    assert x.shape == (B_FULL, T, C), x.shape
    nc = _get_nc()
    ident = np.eye(128, dtype=_BF16_NP)
    in_maps = [{"x": _prep_shard(x[i * B_CORE:(i + 1) * B_CORE]),
                "ident": ident}
               for i in range(N_CORES)]
    res = run_bass_kernel_spmd(nc, in_maps, list(range(N_CORES)),
                               trace=trace, **kw)
    # y[b, type, w, c] -> out[b, w, type*26 + c]
    ys = np.concatenate([res.results[i]["y"] for i in range(N_CORES)], axis=0)
    out = np.concatenate([ys[:, 0], ys[:, 1]], axis=-1)
    return np.ascontiguousarray(out.astype(np.float32)), res


def kernel(x):
    out, _ = _run(x, trace=False)
    return out
